# revision 22
# baseline (speedup 1.0000x reference)
"""DiT attention block on 8 Trainium2 NeuronCores.

Sharding: batch (2) x head-groups (4 heads each) -> 8 cores.
Each core computes, for its batch b and 4 heads:
    q/k/v projections, RMSNorm+rope on q/k, softmax attention, and its
    partial output projection. Host sums the 4 head-group partials per
    batch and transposes back.

Structure (v4):
  - phase 1: QKV projections (f16 matmuls) + RMSNorm + rope, PE-bound.
    RMS sum-of-squares is broadcast across partitions by an all-ones
    [128,128] matmul; rstd = exp(-0.5*ln(ms+eps)) on ScalarE so the
    whole kernel uses one activation table (natural_log_exp).
  - phase 2: attention, q-half-major: for each 1024-wide q block, all
    4 heads. exp over [128,1024] two-bank PSUM tiles. Softmax
    denominator: row-sums accumulated in bf16 on DVE/GpSimd, then an
    all-ones matmul broadcasts the partition sum, 1/den = exp(-ln(den))
    on ScalarE (the exact `reciprocal` costs ~3-5us/call).
  - phase 3 (output projection) for q-block 0 is interleaved into the
    emission of q-block 1's attention, filling TensorE during the
    ACT-bound softmax stretch. Output staged in bf16, one DMA per
    [128, 1024] chunk.
  - fp8 was evaluated and rejected: this problem's rope uses random
    (non-orthogonal) 2x2 mixes, scores reach +-22, and softmax logits
    need <0.3% error; fp8 q/k gives ~10% output error (V-only ~2.9%,
    Wo-only ~3.6%, all past the 2e-2 gate).
"""

import math

import ml_dtypes
import numpy as np

import concourse.bass as bass
import concourse.mybir as mybir
import concourse.tile as tile
from concourse.bass_utils import run_bass_kernel_spmd

F32 = mybir.dt.float32
F16 = mybir.dt.float16
BF16 = mybir.dt.bfloat16
P = 128          # partitions / head_dim
S = 2048         # sequence
D = 2048         # model dim
HD = 128         # head dim
NH = 16          # total heads
NHL = 4          # heads per core
IL = NHL * HD    # 512, inner slice per core
KO = D // P      # 16 contraction tiles
SC = 512         # x-chunk columns in the QKV phase
NSC = S // SC    # 4
QB = 1024        # q-block in attention phase
NQB = S // QB    # 2
QC = 512         # q-chunk in output projection
NQC = QB // QC   # 2 per q-block
NDT = D // P     # 16 output row blocks
EPS = 1e-6
SCALE = 1.0 / math.sqrt(HD)
EXP_BIAS = -4.0 * math.log(2.0)   # exp(x + b); cancels in normalization
N_CORES = 8

Act = mybir.ActivationFunctionType

_PROG_CACHE = {}


def _split_multi_waits(nc, max_waits=1):
    """walrus here rejects >1 sync-wait per instruction; move extras onto
    same-engine nops placed immediately before the instruction."""
    n_split = 0
    for fn in nc.m.functions:
        for bb in fn.blocks:
            insts = bb.instructions
            new_list = []
            changed = False
            for inst in insts:
                si = getattr(inst, "sync_info", None)
                waits = list(si.on_wait) if (si is not None and si.on_wait) else []
                if len(waits) > max_waits:
                    extra = waits[:-max_waits]
                    keep = waits[-max_waits:]
                    for i in range(0, len(extra), max_waits):
                        nop = mybir.InstNoOp(
                            name=f"I-wsplit-{nc.next_id()}", ins=[], outs=[])
                        nop.engine = inst.engine
                        nop.sync_info = mybir.SyncInfo(
                            on_wait=extra[i:i + max_waits], on_update=[])
                        new_list.append(nop)
                        n_split += 1
                    del si.on_wait[:]
                    si.on_wait.extend(keep)
                    changed = True
                new_list.append(inst)
            if changed:
                del insts[:]
                insts.extend(new_list)
    return n_split


def _emit(nc, tc, t, phases=(1, 2, 3)):
    from contextlib import ExitStack

    with ExitStack() as top:
        top.enter_context(nc.allow_low_precision(
            reason="f16/bf16 operands; fp32 accumulation where it matters"))
        const = top.enter_context(tc.tile_pool(name="const", bufs=1))

        pswap = const.tile([P, P], F16, tag="pswap")
        nc.sync.dma_start(pswap[:], t["pswap"][:, :])
        ones_mat = const.tile([P, P], BF16, tag="ones_mat")
        nc.vector.memset(ones_mat, 1.0)
        eps_t = const.tile([P, 1], F32, tag="eps_t")
        nc.vector.memset(eps_t, EPS)
        ebias_t = const.tile([P, 1], F32, tag="ebias_t")
        nc.vector.memset(ebias_t, EXP_BIAS)
        rope = {nm: const.tile([P, S], F16, tag=nm, name=nm)
                for nm in ("ropeAq", "ropeBq", "ropeAk", "ropeBk")}

        # SBUF-resident q/k (transposed per head), V, and attn output
        resid = top.enter_context(tc.tile_pool(name="resid", bufs=1))
        qres = [resid.tile([P, S], F16, tag=f"qres{h}", name=f"qres{h}")
                for h in range(NHL)]
        kres = [resid.tile([P, S], F16, tag=f"kres{h}", name=f"kres{h}")
                for h in range(NHL)]
        vres = resid.tile([P, KO, IL], F16, tag="vres")
        avn = [resid.tile([P, S], F16, tag=f"avn{h}", name=f"avn{h}")
               for h in range(NHL)]

        # ------------------------------------------------------------------
        # Unified emission: phase 1 (QKV+norm+rope), attention, and output
        # projection share one PSUM layout so their instruction streams can
        # interleave:
        #   psA   2 x [P,QB] f32 (4 banks): scores tiles; p1 main
        #         projection accumulators during the dense phase-1 part
        #   psPav 1 x [P,QB] f32 (2 banks): attention AV accumulator
        #   psB   2 x [P,SC] f32 (2 banks): ssq/pswap/denominator/out-proj
        # Emission order:
        #   chunks 0-2: V + all 8 q/k tiles (PE-dense)
        #   chunk 3:    V + the 4 k tiles, then attention on q block 0
        #               cascades in, interleaved with the remaining q tiles
        #   q block 1:  attention interleaved with q-block-0 output chunks
        #   tail:       q-block-1 output chunks
        # ------------------------------------------------------------------
        from contextlib import ExitStack as _ES
        ph = top.enter_context(_ES())
        wpool = ph.enter_context(tc.tile_pool(name="w", bufs=1))
        xpool = ph.enter_context(tc.tile_pool(name="x", bufs=2))
        tp = ph.enter_context(tc.tile_pool(name="qkvtmp", bufs=3))
        tps = ph.enter_context(tc.tile_pool(name="qkvtmps", bufs=2))
        tp2 = ph.enter_context(tc.tile_pool(name="qkvtmp32", bufs=2))
        atp = top.enter_context(tc.tile_pool(name="attnT", bufs=3))
        accp = top.enter_context(tc.tile_pool(name="acc", bufs=2))
        smt = top.enter_context(tc.tile_pool(name="smallt", bufs=2))
        wop = top.enter_context(tc.tile_pool(name="wo", bufs=2))
        otp = top.enter_context(tc.tile_pool(name="ot", bufs=2))
        psA = top.enter_context(tc.tile_pool(name="psA", bufs=3, space="PSUM"))
        psB = top.enter_context(tc.tile_pool(name="psB", bufs=2, space="PSUM"))

        # DMA order matters for startup latency: V weights + first x chunk
        # first (V matmuls are the first PE work), split in halves so the
        # first matmuls can start sooner.
        wv = wpool.tile([P, KO, IL], F16, tag="wv")
        wvT_r = t["wvT"][:, :].rearrange("(ko p) i -> p ko i", p=P)
        xT_r = t["xT"][:, :].rearrange("(ko p) s -> p ko s", p=P)
        xt0 = xpool.tile([P, KO, SC], F16, tag="xchunk")
        nc.sync.dma_start(wv[:, 0:KO // 2, :], wvT_r[:, 0:KO // 2, :])
        nc.sync.dma_start(wv[:, KO // 2:, :], wvT_r[:, KO // 2:, :])
        nc.sync.dma_start(xt0[:, 0:KO // 2, :], xT_r[:, 0:KO // 2, 0:SC])
        nc.sync.dma_start(xt0[:, KO // 2:, :], xT_r[:, KO // 2:, 0:SC])
        wq = wpool.tile([P, KO, IL], F16, tag="wq")
        nc.sync.dma_start(wq[:], t["wqT"][:, :].rearrange("(ko p) i -> p ko i", p=P))
        wk = wpool.tile([P, KO, IL], F16, tag="wk")
        nc.sync.dma_start(wk[:], t["wkT"][:, :].rearrange("(ko p) i -> p ko i", p=P))
        for nm in ("ropeAq", "ropeBq", "ropeAk", "ropeBk"):
            nc.sync.dma_start(rope[nm][:], t[nm][:, :])

        # PE warmup during the initial DMA wait (junk matmuls on memset
        # data): gets the HAM clock gate to full rate before real work.
        warm = psB.tile([P, SC], F32, tag="psB", name="warm")
        for _ in range(8):
            nc.tensor.matmul(warm[:, :P], lhsT=ones_mat[:],
                             rhs=ones_mat[:], start=True, stop=True)

        def v_tile(xt, sc, st, pool, tag):
            pv = pool.tile([P, SC], F32, tag=tag, name="pv")
            for kk in range(KO):
                nc.tensor.matmul(
                    pv[:, :],
                    lhsT=xt[:, kk, st * P:(st + 1) * P],
                    rhs=wv[:, kk, :],
                    start=(kk == 0), stop=(kk == KO - 1))
            nc.scalar.copy(vres[:, sc * (SC // P) + st, :], pv[:, :])

        def qk_tile(xt, sc, wt, ra, rb, dst, h, pool, tag):
            pqk = pool.tile([P, SC], F32, tag=tag, name="pqk")
            for kk in range(KO):
                nc.tensor.matmul(
                    pqk[:, :SC],
                    lhsT=wt[:, kk, h * P:(h + 1) * P],
                    rhs=xt[:, kk, :],
                    start=(kk == 0), stop=(kk == KO - 1))
            raw = tp.tile([P, SC], F16, tag="raw")
            nc.scalar.copy(raw[:], pqk[:, :SC])
            # sum of squares over head_dim, broadcast by all-ones matmul;
            # rstd = exp(-0.5*ln(ms+eps)) keeps everything in one ACT table
            sq = tps.tile([P, SC], BF16, tag="sq")
            nc.gpsimd.tensor_mul(sq[:], raw[:], raw[:])
            pssq = psB.tile([P, SC], F32, tag="psB", name="pssq")
            nc.tensor.matmul(pssq[:], lhsT=ones_mat[:], rhs=sq[:],
                             start=True, stop=True)
            lt = tp2.tile([P, SC], F32, tag="lnms")
            nc.scalar.activation(lt[:], pssq[:], func=Act.Ln,
                                 bias=eps_t[:], scale=1.0 / HD)
            rstd = tp.tile([P, SC], F16, tag="rstd")
            nc.scalar.activation(rstd[:], lt[:], func=Act.Exp, scale=-0.5)
            # rope rotate-half swap via permutation matmul
            psw = psB.tile([P, SC], F32, tag="psB", name="psw")
            nc.tensor.matmul(psw[:], lhsT=pswap[:], rhs=raw[:],
                             start=True, stop=True)
            tmp = tps.tile([P, SC], F16, tag="ropetmp")
            nc.vector.tensor_mul(tmp[:], ra[:, sc * SC:(sc + 1) * SC], raw[:])
            tmp2 = tps.tile([P, SC], F16, tag="ropetmp2")
            nc.vector.tensor_mul(tmp2[:], rb[:, sc * SC:(sc + 1) * SC], psw[:])
            roped = tps.tile([P, SC], F16, tag="roped")
            nc.vector.tensor_add(roped[:], tmp[:], tmp2[:])
            nc.vector.tensor_mul(
                dst[h][:, sc * SC:(sc + 1) * SC], roped[:], rstd[:])

        QSET = (wq, rope["ropeAq"], rope["ropeBq"], qres)
        KSET = (wk, rope["ropeAk"], rope["ropeBk"], kres)

        def att_steps(h, q0, qw):
            """Attention for head h over q columns [q0, q0+qw); yields
            after each kt step so other work can interleave."""
            nh = qw // QC
            pav = psA.tile([P, qw], F32, tag="psA", name="pav")
            acc0 = accp.tile([P, qw], BF16, tag="acc0", name="acc0")
            acc1 = accp.tile([P, qw], BF16, tag="acc1", name="acc1")

            for kt in range(KO):
                psc = psA.tile([P, qw], F32, tag="psA", name="psc")
                for half in range(nh):
                    nc.tensor.matmul(
                        psc[:, half * QC:(half + 1) * QC],
                        lhsT=kres[h][:, kt * P:(kt + 1) * P],
                        rhs=qres[h][:, q0 + half * QC:q0 + (half + 1) * QC],
                        start=True, stop=True)
                at = atp.tile([P, qw], BF16, tag="at", name="at")
                nc.scalar.activation(at[:], psc[:], func=Act.Exp,
                                     bias=ebias_t[:], scale=SCALE)
                for half in range(nh):
                    nc.tensor.matmul(
                        pav[:, half * QC:(half + 1) * QC],
                        lhsT=vres[:, kt, h * HD:(h + 1) * HD],
                        rhs=at[:, half * QC:(half + 1) * QC],
                        start=(kt == 0), stop=(kt == KO - 1))
                # running row-sum accumulation split DVE/GpSimd
                if kt == 0:
                    nc.vector.tensor_copy(acc0[:], at[:])
                elif kt == 1:
                    nc.gpsimd.tensor_copy(acc1[:], at[:])
                elif kt % 2 == 0:
                    nc.vector.tensor_add(acc0[:], acc0[:], at[:])
                elif kt in (3, 9, 15):
                    nc.gpsimd.tensor_add(acc1[:], acc1[:], at[:])
                else:
                    nc.vector.tensor_add(acc1[:], acc1[:], at[:])
                yield

            # denominator: two accumulated all-ones matmuls (acc0 + acc1)
            # broadcast the partition sum; 1/den via exp(-ln(den))
            for half in range(nh):
                prb = psB.tile([P, QC], F32, tag="psB", name="prb")
                nc.tensor.matmul(
                    prb[:], lhsT=ones_mat[:],
                    rhs=acc0[:, half * QC:(half + 1) * QC],
                    start=True, stop=False)
                nc.tensor.matmul(
                    prb[:], lhsT=ones_mat[:],
                    rhs=acc1[:, half * QC:(half + 1) * QC],
                    start=False, stop=True)
                ldn = smt.tile([P, QC], F32, tag="ldn")
                nc.scalar.activation(ldn[:], prb[:], func=Act.Ln)
                rbs = smt.tile([P, QC], F32, tag="rbs")
                nc.scalar.activation(rbs[:], ldn[:], func=Act.Exp,
                                     scale=-1.0)
                nc.vector.tensor_mul(
                    avn[h][:, q0 + half * QC:q0 + (half + 1) * QC],
                    pav[:, half * QC:(half + 1) * QC], rbs[:])
            yield

        woT_r = t["woT"][:, :].rearrange("(it p) d -> p it d", p=P)

        def p3_chunk(dt, q0, qw):
            """Output projection rows [dt*128,(dt+1)*128) x q columns
            [q0, q0+qw); bf16-staged, one DMA."""
            wo_t = wop.tile([P, NHL, P], F16, tag="wo_t", name="wo_t")
            nc.sync.dma_start(wo_t[:], woT_r[:, :, dt * P:(dt + 1) * P])
            ot = otp.tile([P, qw], BF16, tag="ot", name="ot")
            for qc in range(qw // QC):
                po = psB.tile([P, QC], F32, tag="psB", name="po")
                for it in range(NHL):
                    nc.tensor.matmul(
                        po[:],
                        lhsT=wo_t[:, it, :],
                        rhs=avn[it][:, q0 + qc * QC:q0 + (qc + 1) * QC],
                        start=(it == 0), stop=(it == NHL - 1))
                nc.vector.tensor_copy(ot[:, qc * QC:(qc + 1) * QC], po[:])
            nc.sync.dma_start(
                t["outT"][dt * P:(dt + 1) * P, q0:q0 + qw], ot[:])

        # ---- chunks 0-2: dense phase 1 ----
        for sc in range(NSC - 1):
            if sc == 0:
                xt = xt0
            else:
                xt = xpool.tile([P, KO, SC], F16, tag="xchunk")
                nc.sync.dma_start(xt[:], xT_r[:, :, sc * SC:(sc + 1) * SC])
            for st in range(SC // P):
                v_tile(xt, sc, st, psA, "psA")
            for wt, ra, rb, dst in (QSET, KSET):
                for h in range(NHL):
                    qk_tile(xt, sc, wt, ra, rb, dst, h, psA, "psA")

        # ---- chunk 3: V + k tiles, then q-block-0 attention cascades in,
        # interleaved with the remaining q tiles ----
        sc = NSC - 1
        xt3 = xpool.tile([P, KO, SC], F16, tag="xchunk")
        nc.sync.dma_start(xt3[:], xT_r[:, :, sc * SC:(sc + 1) * SC])
        for st in range(SC // P):
            v_tile(xt3, sc, st, psB, "psB")
        for h in range(NHL):
            qk_tile(xt3, sc, *KSET, h, psB, "psB")

        pending_q = list(range(NHL))
        for h in range(NHL):
            for i, _ in enumerate(att_steps(h, 0, QB)):
                if i in (4, 10) and pending_q:
                    qk_tile(xt3, sc, *QSET, pending_q.pop(0), psB, "psB")
        for h in pending_q:
            qk_tile(xt3, sc, *QSET, h, psB, "psB")

        # ---- q block 1: attention interleaved with q-block-0 out-proj ----
        dt_iter = iter(range(NDT))
        for h in range(NHL):
            for i, _ in enumerate(att_steps(h, QB, QB)):
                if i % 4 == 3:
                    dt = next(dt_iter, None)
                    if dt is not None:
                        p3_chunk(dt, 0, QB)
        for dt in dt_iter:
            p3_chunk(dt, 0, QB)
        # ---- q block 1 output projection tail ----
        for dt in range(NDT):
            p3_chunk(dt, QB, QB)


def _build_program(loop_n=0, phases=(1, 2, 3)):
    key = ("nc", loop_n, tuple(phases))
    if key in _PROG_CACHE:
        return _PROG_CACHE[key]
    nc = bass.Bass()
    t = {}
    t["xT"] = nc.dram_tensor("xT", [D, S], F16, kind="ExternalInput")
    t["wqT"] = nc.dram_tensor("wqT", [D, IL], F16, kind="ExternalInput")
    t["wkT"] = nc.dram_tensor("wkT", [D, IL], F16, kind="ExternalInput")
    t["wvT"] = nc.dram_tensor("wvT", [D, IL], F16, kind="ExternalInput")
    t["woT"] = nc.dram_tensor("woT", [IL, D], F16, kind="ExternalInput")
    for nm in ("ropeAq", "ropeBq", "ropeAk", "ropeBk"):
        t[nm] = nc.dram_tensor(nm, [P, S], F16, kind="ExternalInput")
    t["pswap"] = nc.dram_tensor("pswap", [P, P], F16, kind="ExternalInput")
    t["outT"] = nc.dram_tensor("outT", [D, S], BF16, kind="ExternalOutput")

    with tile.TileContext(nc) as tc:
        if loop_n:
            with tc.For_i(0, loop_n):
                _emit(nc, tc, t, phases)
        else:
            _emit(nc, tc, t, phases)
    _split_multi_waits(nc)
    _PROG_CACHE[key] = nc
    return nc


def _prep_in_maps(x, rope_emb, Wq, Wk, Wv, Wo, q_norm_w, k_norm_w):
    x = np.asarray(x, np.float32)
    F = np.asarray(rope_emb, np.float32)[:, 0]          # [S, 64, 2, 2]
    A0 = np.concatenate([F[:, :, 0, 0], F[:, :, 1, 1]], axis=-1)  # [S, 128]
    B0 = np.concatenate([F[:, :, 0, 1], F[:, :, 1, 0]], axis=-1)  # [S, 128]

    def rope_consts(w):
        w = np.asarray(w, np.float32)
        w_sw = np.concatenate([w[64:], w[:64]])
        A = np.ascontiguousarray((A0 * w[None, :]).T)    # [128, S]
        B = np.ascontiguousarray((B0 * w_sw[None, :]).T)
        return A.astype(np.float16), B.astype(np.float16)

    Aq, Bq = rope_consts(q_norm_w)
    Ak, Bk = rope_consts(k_norm_w)
    pswap = np.zeros((P, P), np.float16)
    for d in range(P):
        pswap[(d + 64) % P, d] = 1.0

    bf = np.float16
    xT = [np.ascontiguousarray(x[b].T).astype(bf) for b in range(x.shape[0])]
    Wq = np.asarray(Wq, np.float32)
    Wk = np.asarray(Wk, np.float32)
    Wv = np.asarray(Wv, np.float32)
    Wo = np.asarray(Wo, np.float32)

    in_maps = []
    for c in range(N_CORES):
        b, hg = divmod(c, NH // NHL)
        sl = slice(hg * IL, (hg + 1) * IL)
        in_maps.append({
            "xT": xT[b],
            "wqT": np.ascontiguousarray(Wq[sl, :].T).astype(bf),
            "wkT": np.ascontiguousarray(Wk[sl, :].T).astype(bf),
            "wvT": np.ascontiguousarray(Wv[sl, :].T).astype(bf),
            "woT": np.ascontiguousarray(Wo[:, sl].T).astype(bf),
            "ropeAq": Aq, "ropeBq": Bq, "ropeAk": Ak, "ropeBk": Bk,
            "pswap": pswap,
        })
    return in_maps


def kernel(x, rope_emb, Wq, Wk, Wv, Wo, q_norm_w, k_norm_w, _trace=False):
    nc = _build_program()
    in_maps = _prep_in_maps(x, rope_emb, Wq, Wk, Wv, Wo, q_norm_w, k_norm_w)
    res = run_bass_kernel_spmd(nc, in_maps, core_ids=list(range(N_CORES)),
                               trace=_trace)
    out = np.empty((2, S, D), np.float32)
    for b in range(2):
        acc = res.results[4 * b]["outT"].astype(np.float32)
        for hg in range(1, 4):
            acc += res.results[4 * b + hg]["outT"].astype(np.float32)
        out[b] = acc.T
    if _trace:
        kernel.last_exec_time_ns = res.exec_time_ns
        kernel.last_results = res
    return out


# revision 24
# speedup vs baseline: 1.0466x; 1.0466x over previous
"""DiT attention block on 8 Trainium2 NeuronCores.

Sharding: batch (2) x head-groups (4 heads each) -> 8 cores.
Each core computes, for its batch b and 4 heads:
    q/k/v projections, RMSNorm+rope on q/k, softmax attention, and its
    partial output projection. Host sums the 4 head-group partials per
    batch and transposes back.

Structure (v4):
  - phase 1: QKV projections (f16 matmuls) + RMSNorm + rope, PE-bound.
    RMS sum-of-squares is broadcast across partitions by an all-ones
    [128,128] matmul; rstd = exp(-0.5*ln(ms+eps)) on ScalarE so the
    whole kernel uses one activation table (natural_log_exp).
  - phase 2: attention, q-half-major: for each 1024-wide q block, all
    4 heads. exp over [128,1024] two-bank PSUM tiles. Softmax
    denominator: row-sums accumulated in bf16 on DVE/GpSimd, then an
    all-ones matmul broadcasts the partition sum, 1/den = exp(-ln(den))
    on ScalarE (the exact `reciprocal` costs ~3-5us/call).
  - phase 3 (output projection) for q-block 0 is interleaved into the
    emission of q-block 1's attention, filling TensorE during the
    ACT-bound softmax stretch. Output staged in bf16, one DMA per
    [128, 1024] chunk.
  - fp8 was evaluated and rejected: this problem's rope uses random
    (non-orthogonal) 2x2 mixes, scores reach +-22, and softmax logits
    need <0.3% error; fp8 q/k gives ~10% output error (V-only ~2.9%,
    Wo-only ~3.6%, all past the 2e-2 gate).
"""

import math

import ml_dtypes
import numpy as np

import concourse.bass as bass
import concourse.mybir as mybir
import concourse.tile as tile
from concourse.bass_utils import run_bass_kernel_spmd

F32 = mybir.dt.float32
F16 = mybir.dt.float16
BF16 = mybir.dt.bfloat16
P = 128          # partitions / head_dim
S = 2048         # sequence
D = 2048         # model dim
HD = 128         # head dim
NH = 16          # total heads
NHL = 4          # heads per core
IL = NHL * HD    # 512, inner slice per core
KO = D // P      # 16 contraction tiles
SC = 512         # x-chunk columns in the QKV phase
NSC = S // SC    # 4
QB = 1024        # q-block in attention phase
NQB = S // QB    # 2
QC = 512         # q-chunk in output projection
NQC = QB // QC   # 2 per q-block
NDT = D // P     # 16 output row blocks
EPS = 1e-6
SCALE = 1.0 / math.sqrt(HD)
EXP_BIAS = -4.0 * math.log(2.0)   # exp(x + b); cancels in normalization
N_CORES = 8

Act = mybir.ActivationFunctionType

_PROG_CACHE = {}


def _split_multi_waits(nc, max_waits=1):
    """walrus here rejects >1 sync-wait per instruction; move extras onto
    same-engine nops placed immediately before the instruction."""
    n_split = 0
    for fn in nc.m.functions:
        for bb in fn.blocks:
            insts = bb.instructions
            new_list = []
            changed = False
            for inst in insts:
                si = getattr(inst, "sync_info", None)
                waits = list(si.on_wait) if (si is not None and si.on_wait) else []
                if len(waits) > max_waits:
                    extra = waits[:-max_waits]
                    keep = waits[-max_waits:]
                    for i in range(0, len(extra), max_waits):
                        nop = mybir.InstNoOp(
                            name=f"I-wsplit-{nc.next_id()}", ins=[], outs=[])
                        nop.engine = inst.engine
                        nop.sync_info = mybir.SyncInfo(
                            on_wait=extra[i:i + max_waits], on_update=[])
                        new_list.append(nop)
                        n_split += 1
                    del si.on_wait[:]
                    si.on_wait.extend(keep)
                    changed = True
                new_list.append(inst)
            if changed:
                del insts[:]
                insts.extend(new_list)
    return n_split


def _emit(nc, tc, t, phases=(1, 2, 3)):
    from contextlib import ExitStack

    with ExitStack() as top:
        top.enter_context(nc.allow_low_precision(
            reason="f16/bf16 operands; fp32 accumulation where it matters"))
        const = top.enter_context(tc.tile_pool(name="const", bufs=1))

        pswap = const.tile([P, P], F16, tag="pswap")
        nc.sync.dma_start(pswap[:], t["pswap"][:, :])
        ones_mat = const.tile([P, P], BF16, tag="ones_mat")
        nc.vector.memset(ones_mat, 1.0)
        eps_t = const.tile([P, 1], F32, tag="eps_t")
        nc.vector.memset(eps_t, EPS)
        ebias_t = const.tile([P, 1], F32, tag="ebias_t")
        nc.vector.memset(ebias_t, EXP_BIAS)
        rope = {nm: const.tile([P, S], F16, tag=nm, name=nm)
                for nm in ("ropeAq", "ropeBq", "ropeAk", "ropeBk")}

        # SBUF-resident q/k (transposed per head), V, and attn output
        resid = top.enter_context(tc.tile_pool(name="resid", bufs=1))
        qres = [resid.tile([P, S], F16, tag=f"qres{h}", name=f"qres{h}")
                for h in range(NHL)]
        kres = [resid.tile([P, S], F16, tag=f"kres{h}", name=f"kres{h}")
                for h in range(NHL)]
        vres = resid.tile([P, KO, IL], F16, tag="vres")
        avn = [resid.tile([P, S], F16, tag=f"avn{h}", name=f"avn{h}")
               for h in range(NHL)]

        # ------------------------------------------------------------------
        # Unified emission: phase 1 (QKV+norm+rope), attention, and output
        # projection share one PSUM layout so their instruction streams can
        # interleave:
        #   psA   2 x [P,QB] f32 (4 banks): scores tiles; p1 main
        #         projection accumulators during the dense phase-1 part
        #   psPav 1 x [P,QB] f32 (2 banks): attention AV accumulator
        #   psB   2 x [P,SC] f32 (2 banks): ssq/pswap/denominator/out-proj
        # Emission order:
        #   chunks 0-2: V + all 8 q/k tiles (PE-dense)
        #   chunk 3:    V + the 4 k tiles, then attention on q block 0
        #               cascades in, interleaved with the remaining q tiles
        #   q block 1:  attention interleaved with q-block-0 output chunks
        #   tail:       q-block-1 output chunks
        # ------------------------------------------------------------------
        from contextlib import ExitStack as _ES
        ph = top.enter_context(_ES())
        wpool = ph.enter_context(tc.tile_pool(name="w", bufs=1))
        xpool = ph.enter_context(tc.tile_pool(name="x", bufs=2))
        tp = ph.enter_context(tc.tile_pool(name="qkvtmp", bufs=3))
        tps = ph.enter_context(tc.tile_pool(name="qkvtmps", bufs=2))
        tp2 = ph.enter_context(tc.tile_pool(name="qkvtmp32", bufs=2))
        atp = top.enter_context(tc.tile_pool(name="attnT", bufs=3))
        accp = top.enter_context(tc.tile_pool(name="acc", bufs=2))
        smt = top.enter_context(tc.tile_pool(name="smallt", bufs=2))
        wop = top.enter_context(tc.tile_pool(name="wo", bufs=2))
        otp = top.enter_context(tc.tile_pool(name="ot", bufs=2))
        psA = top.enter_context(tc.tile_pool(name="psA", bufs=2, space="PSUM"))
        psPav = top.enter_context(
            tc.tile_pool(name="psPav", bufs=1, space="PSUM"))
        psB = top.enter_context(tc.tile_pool(name="psB", bufs=2, space="PSUM"))

        # DMA order matters for startup latency: V weights + first x chunk
        # first (V matmuls are the first PE work), split in halves so the
        # first matmuls can start sooner.
        wv = wpool.tile([P, KO, IL], F16, tag="wv")
        wvT_r = t["wvT"][:, :].rearrange("(ko p) i -> p ko i", p=P)
        xT_r = t["xT"][:, :].rearrange("(ko p) s -> p ko s", p=P)
        xt0 = xpool.tile([P, KO, SC], F16, tag="xchunk")
        for qtr in range(4):
            k0, k1 = qtr * (KO // 4), (qtr + 1) * (KO // 4)
            nc.sync.dma_start(wv[:, k0:k1, :], wvT_r[:, k0:k1, :])
            nc.sync.dma_start(xt0[:, k0:k1, :], xT_r[:, k0:k1, 0:SC])
        wq = wpool.tile([P, KO, IL], F16, tag="wq")
        nc.sync.dma_start(wq[:], t["wqT"][:, :].rearrange("(ko p) i -> p ko i", p=P))
        wk = wpool.tile([P, KO, IL], F16, tag="wk")
        nc.sync.dma_start(wk[:], t["wkT"][:, :].rearrange("(ko p) i -> p ko i", p=P))
        for nm in ("ropeAq", "ropeBq", "ropeAk", "ropeBk"):
            nc.sync.dma_start(rope[nm][:], t[nm][:, :])

        # PE warmup during the initial DMA wait (junk matmuls on memset
        # data): gets the HAM clock gate to full rate before real work.
        warm = psB.tile([P, SC], F32, tag="psB", name="warm")
        for _ in range(8):
            nc.tensor.matmul(warm[:, :P], lhsT=ones_mat[:],
                             rhs=ones_mat[:], start=True, stop=True)

        def v_tile(xt, sc, st, pool, tag):
            pv = pool.tile([P, SC], F32, tag=tag, name="pv")
            for kk in range(KO):
                nc.tensor.matmul(
                    pv[:, :],
                    lhsT=xt[:, kk, st * P:(st + 1) * P],
                    rhs=wv[:, kk, :],
                    start=(kk == 0), stop=(kk == KO - 1))
            nc.scalar.copy(vres[:, sc * (SC // P) + st, :], pv[:, :])

        def qk_tile(xt, sc, wt, ra, rb, dst, h, pool, tag):
            pqk = pool.tile([P, SC], F32, tag=tag, name="pqk")
            for kk in range(KO):
                nc.tensor.matmul(
                    pqk[:, :SC],
                    lhsT=wt[:, kk, h * P:(h + 1) * P],
                    rhs=xt[:, kk, :],
                    start=(kk == 0), stop=(kk == KO - 1))
            raw = tp.tile([P, SC], F16, tag="raw")
            nc.scalar.copy(raw[:], pqk[:, :SC])
            # sum of squares over head_dim, broadcast by all-ones matmul;
            # rstd = exp(-0.5*ln(ms+eps)) keeps everything in one ACT table
            sq = tps.tile([P, SC], BF16, tag="sq")
            nc.gpsimd.tensor_mul(sq[:], raw[:], raw[:])
            pssq = psB.tile([P, SC], F32, tag="psB", name="pssq")
            nc.tensor.matmul(pssq[:], lhsT=ones_mat[:], rhs=sq[:],
                             start=True, stop=True)
            lt = tp2.tile([P, SC], F32, tag="lnms")
            nc.scalar.activation(lt[:], pssq[:], func=Act.Ln,
                                 bias=eps_t[:], scale=1.0 / HD)
            rstd = tp.tile([P, SC], F16, tag="rstd")
            nc.scalar.activation(rstd[:], lt[:], func=Act.Exp, scale=-0.5)
            # rope rotate-half swap via permutation matmul
            psw = psB.tile([P, SC], F32, tag="psB", name="psw")
            nc.tensor.matmul(psw[:], lhsT=pswap[:], rhs=raw[:],
                             start=True, stop=True)
            tmp = tps.tile([P, SC], F16, tag="ropetmp")
            nc.vector.tensor_mul(tmp[:], ra[:, sc * SC:(sc + 1) * SC], raw[:])
            tmp2 = tps.tile([P, SC], F16, tag="ropetmp2")
            nc.vector.tensor_mul(tmp2[:], rb[:, sc * SC:(sc + 1) * SC], psw[:])
            roped = tps.tile([P, SC], F16, tag="roped")
            nc.vector.tensor_add(roped[:], tmp[:], tmp2[:])
            nc.vector.tensor_mul(
                dst[h][:, sc * SC:(sc + 1) * SC], roped[:], rstd[:])

        QSET = (wq, rope["ropeAq"], rope["ropeBq"], qres)
        KSET = (wk, rope["ropeAk"], rope["ropeBk"], kres)

        def att_steps(h, q0, qw):
            """Attention for head h over q columns [q0, q0+qw); yields
            after each kt step so other work can interleave."""
            nh = qw // QC
            pav = psPav.tile([P, qw], F32, tag="psPav", name="pav")
            acc0 = accp.tile([P, qw], BF16, tag="acc0", name="acc0")
            acc1 = accp.tile([P, qw], BF16, tag="acc1", name="acc1")

            for kt in range(KO):
                psc = psA.tile([P, qw], F32, tag="psA", name="psc")
                for half in range(nh):
                    nc.tensor.matmul(
                        psc[:, half * QC:(half + 1) * QC],
                        lhsT=kres[h][:, kt * P:(kt + 1) * P],
                        rhs=qres[h][:, q0 + half * QC:q0 + (half + 1) * QC],
                        start=True, stop=True)
                at = atp.tile([P, qw], BF16, tag="at", name="at")
                nc.scalar.activation(at[:], psc[:], func=Act.Exp,
                                     bias=ebias_t[:], scale=SCALE)
                for half in range(nh):
                    nc.tensor.matmul(
                        pav[:, half * QC:(half + 1) * QC],
                        lhsT=vres[:, kt, h * HD:(h + 1) * HD],
                        rhs=at[:, half * QC:(half + 1) * QC],
                        start=(kt == 0), stop=(kt == KO - 1))
                # running row-sum accumulation split DVE/GpSimd
                if kt == 0:
                    nc.vector.tensor_copy(acc0[:], at[:])
                elif kt == 1:
                    nc.gpsimd.tensor_copy(acc1[:], at[:])
                elif kt % 2 == 0:
                    nc.vector.tensor_add(acc0[:], acc0[:], at[:])
                elif kt in (3, 9, 15):
                    nc.gpsimd.tensor_add(acc1[:], acc1[:], at[:])
                else:
                    nc.vector.tensor_add(acc1[:], acc1[:], at[:])
                yield

            # denominator: two accumulated all-ones matmuls (acc0 + acc1)
            # broadcast the partition sum; 1/den via exp(-ln(den))
            for half in range(nh):
                prb = psB.tile([P, QC], F32, tag="psB", name="prb")
                nc.tensor.matmul(
                    prb[:], lhsT=ones_mat[:],
                    rhs=acc0[:, half * QC:(half + 1) * QC],
                    start=True, stop=False)
                nc.tensor.matmul(
                    prb[:], lhsT=ones_mat[:],
                    rhs=acc1[:, half * QC:(half + 1) * QC],
                    start=False, stop=True)
                ldn = smt.tile([P, QC], F32, tag="ldn")
                nc.scalar.activation(ldn[:], prb[:], func=Act.Ln)
                rbs = smt.tile([P, QC], F32, tag="rbs")
                nc.scalar.activation(rbs[:], ldn[:], func=Act.Exp,
                                     scale=-1.0)
                nc.vector.tensor_mul(
                    avn[h][:, q0 + half * QC:q0 + (half + 1) * QC],
                    pav[:, half * QC:(half + 1) * QC], rbs[:])
            yield

        woT_r = t["woT"][:, :].rearrange("(it p) d -> p it d", p=P)

        def p3_chunk(dt, q0, qw):
            """Output projection rows [dt*128,(dt+1)*128) x q columns
            [q0, q0+qw); bf16-staged, one DMA."""
            wo_t = wop.tile([P, NHL, P], F16, tag="wo_t", name="wo_t")
            nc.sync.dma_start(wo_t[:], woT_r[:, :, dt * P:(dt + 1) * P])
            ot = otp.tile([P, qw], BF16, tag="ot", name="ot")
            for qc in range(qw // QC):
                po = psB.tile([P, QC], F32, tag="psB", name="po")
                for it in range(NHL):
                    nc.tensor.matmul(
                        po[:],
                        lhsT=wo_t[:, it, :],
                        rhs=avn[it][:, q0 + qc * QC:q0 + (qc + 1) * QC],
                        start=(it == 0), stop=(it == NHL - 1))
                nc.vector.tensor_copy(ot[:, qc * QC:(qc + 1) * QC], po[:])
            nc.sync.dma_start(
                t["outT"][dt * P:(dt + 1) * P, q0:q0 + qw], ot[:])

        # ---- chunks 0-2: dense phase 1 ----
        for sc in range(NSC - 1):
            if sc == 0:
                xt = xt0
            else:
                xt = xpool.tile([P, KO, SC], F16, tag="xchunk")
                nc.sync.dma_start(xt[:], xT_r[:, :, sc * SC:(sc + 1) * SC])
            for st in range(SC // P):
                v_tile(xt, sc, st, psA, "psA")
            for wt, ra, rb, dst in (QSET, KSET):
                for h in range(NHL):
                    qk_tile(xt, sc, wt, ra, rb, dst, h, psA, "psA")

        # ---- chunk 3: V + k tiles, then q-block-0 attention cascades in,
        # interleaved with the remaining q tiles ----
        sc = NSC - 1
        xt3 = xpool.tile([P, KO, SC], F16, tag="xchunk")
        nc.sync.dma_start(xt3[:], xT_r[:, :, sc * SC:(sc + 1) * SC])
        for st in range(SC // P):
            v_tile(xt3, sc, st, psB, "psB")
        for h in range(NHL):
            qk_tile(xt3, sc, *KSET, h, psB, "psB")

        pending_q = list(range(NHL))
        for h in range(NHL):
            for i, _ in enumerate(att_steps(h, 0, QB)):
                if i in (4, 10) and pending_q:
                    qk_tile(xt3, sc, *QSET, pending_q.pop(0), psB, "psB")
        for h in pending_q:
            qk_tile(xt3, sc, *QSET, h, psB, "psB")

        # ---- q block 1: attention interleaved with q-block-0 out-proj ----
        dt_iter = iter(range(NDT))
        for h in range(NHL):
            for i, _ in enumerate(att_steps(h, QB, QB)):
                if i % 4 == 3:
                    dt = next(dt_iter, None)
                    if dt is not None:
                        p3_chunk(dt, 0, QB)
        for dt in dt_iter:
            p3_chunk(dt, 0, QB)
        # ---- q block 1 output projection tail ----
        for dt in range(NDT):
            p3_chunk(dt, QB, QB)


def _build_program(loop_n=0, phases=(1, 2, 3)):
    key = ("nc", loop_n, tuple(phases))
    if key in _PROG_CACHE:
        return _PROG_CACHE[key]
    nc = bass.Bass()
    t = {}
    t["xT"] = nc.dram_tensor("xT", [D, S], F16, kind="ExternalInput")
    t["wqT"] = nc.dram_tensor("wqT", [D, IL], F16, kind="ExternalInput")
    t["wkT"] = nc.dram_tensor("wkT", [D, IL], F16, kind="ExternalInput")
    t["wvT"] = nc.dram_tensor("wvT", [D, IL], F16, kind="ExternalInput")
    t["woT"] = nc.dram_tensor("woT", [IL, D], F16, kind="ExternalInput")
    for nm in ("ropeAq", "ropeBq", "ropeAk", "ropeBk"):
        t[nm] = nc.dram_tensor(nm, [P, S], F16, kind="ExternalInput")
    t["pswap"] = nc.dram_tensor("pswap", [P, P], F16, kind="ExternalInput")
    t["outT"] = nc.dram_tensor("outT", [D, S], BF16, kind="ExternalOutput")

    with tile.TileContext(nc) as tc:
        if loop_n:
            with tc.For_i(0, loop_n):
                _emit(nc, tc, t, phases)
        else:
            _emit(nc, tc, t, phases)
    _split_multi_waits(nc)
    _PROG_CACHE[key] = nc
    return nc


def _prep_in_maps(x, rope_emb, Wq, Wk, Wv, Wo, q_norm_w, k_norm_w):
    x = np.asarray(x, np.float32)
    F = np.asarray(rope_emb, np.float32)[:, 0]          # [S, 64, 2, 2]
    A0 = np.concatenate([F[:, :, 0, 0], F[:, :, 1, 1]], axis=-1)  # [S, 128]
    B0 = np.concatenate([F[:, :, 0, 1], F[:, :, 1, 0]], axis=-1)  # [S, 128]

    def rope_consts(w):
        w = np.asarray(w, np.float32)
        w_sw = np.concatenate([w[64:], w[:64]])
        A = np.ascontiguousarray((A0 * w[None, :]).T)    # [128, S]
        B = np.ascontiguousarray((B0 * w_sw[None, :]).T)
        return A.astype(np.float16), B.astype(np.float16)

    Aq, Bq = rope_consts(q_norm_w)
    Ak, Bk = rope_consts(k_norm_w)
    pswap = np.zeros((P, P), np.float16)
    for d in range(P):
        pswap[(d + 64) % P, d] = 1.0

    bf = np.float16
    xT = [np.ascontiguousarray(x[b].T).astype(bf) for b in range(x.shape[0])]
    Wq = np.asarray(Wq, np.float32)
    Wk = np.asarray(Wk, np.float32)
    Wv = np.asarray(Wv, np.float32)
    Wo = np.asarray(Wo, np.float32)

    in_maps = []
    for c in range(N_CORES):
        b, hg = divmod(c, NH // NHL)
        sl = slice(hg * IL, (hg + 1) * IL)
        in_maps.append({
            "xT": xT[b],
            "wqT": np.ascontiguousarray(Wq[sl, :].T).astype(bf),
            "wkT": np.ascontiguousarray(Wk[sl, :].T).astype(bf),
            "wvT": np.ascontiguousarray(Wv[sl, :].T).astype(bf),
            "woT": np.ascontiguousarray(Wo[:, sl].T).astype(bf),
            "ropeAq": Aq, "ropeBq": Bq, "ropeAk": Ak, "ropeBk": Bk,
            "pswap": pswap,
        })
    return in_maps


def kernel(x, rope_emb, Wq, Wk, Wv, Wo, q_norm_w, k_norm_w, _trace=False):
    nc = _build_program()
    in_maps = _prep_in_maps(x, rope_emb, Wq, Wk, Wv, Wo, q_norm_w, k_norm_w)
    res = run_bass_kernel_spmd(nc, in_maps, core_ids=list(range(N_CORES)),
                               trace=_trace)
    out = np.empty((2, S, D), np.float32)
    for b in range(2):
        acc = res.results[4 * b]["outT"].astype(np.float32)
        for hg in range(1, 4):
            acc += res.results[4 * b + hg]["outT"].astype(np.float32)
        out[b] = acc.T
    if _trace:
        kernel.last_exec_time_ns = res.exec_time_ns
        kernel.last_results = res
    return out
